# revision 26
# baseline (speedup 1.0000x reference)
"""Fused sparse-attention CNN kernel for TRN2 (8 NeuronCores, batch-parallel).

Per batch b (one per core), with L=2048, H=128:
  cos[l,m] = <s_l, s_m> / (|s_l||s_m|)  masked to band (m <= l+2, diag removed
  except (0,0)); att = softmax over l (per-column normalization);
  x2 = att @ x; GLU over concat([x, x2, individual]); 3x causal conv1d(K=3)
  + relu; times next_skill.

Key structure exploited on-chip:
  - softmax normalizes over full columns m, so att = E / colsum(E) with
    E = exp(masked cos) and x2 = E^T-layout matmul with x pre-scaled by
    1/colsum. No online softmax needed.
  - E is stored transposed, ET[m, l], so colsums are free-axis reductions
    (fused into the exp pass via accum_out) and both big matmuls contract
    over the partition axis naturally.
  - the band mask means tiles with l < m-2 are skipped entirely (~1/3 of
    the [L, L] work) and only diagonal-adjacent tiles need mask multiplies.
  - everything after the cos matmul stays in SBUF; the [L, L] tensor never
    touches HBM.
"""

import numpy as np

import concourse.bacc as bacc
import concourse.bass as bass
import concourse.tile as tile
from concourse import mybir
from concourse.bass_utils import run_bass_kernel_spmd
from concourse.masks import make_identity

L = 2048
H = 128
NT = 16  # l/m tiles of 128
B = 8
F32 = mybir.dt.float32
F32R = mybir.dt.float32r
AF = mybir.ActivationFunctionType
ALU = mybir.AluOpType
AX = mybir.AxisListType

# last m-tile index contributing to each 512-wide l-chunk of x2
_LAST_I = [4, 8, 12, 15]

PRELOAD_TABLES = True
TTR_NORMS = True
GLU_PER_CHUNK = True


def _c0(i):
    # first 512-chunk of l covered by m-tile i's ET strip
    return min(3, max(0, (128 * i - 2) // 512))


def _p1_chunks(i):
    """Phase-1 cos-matmul chunks for m-tile i: (masked_chunk, bulk_groups).

    masked_chunk: (lo, n, mask_kind); bulk_groups: list of lists of (lo, n)
    where each group (<=2 adjacent chunks, total <=1024) shares one PSUM
    tile and one exp pass.
    """
    mi = 128 * i
    if i == 0:
        mk = (0, 512, "mask0")
        lo = 512
    else:
        mk = (mi - 128, 256, "maskcd")
        lo = mi + 128
    rem = L - lo
    chunks = []
    while rem > 0:
        n = 384 if rem == 640 else (512 if rem >= 512 else rem)
        chunks.append((lo, n))
        lo += n
        rem -= n
    # pair adjacent chunks into one 2-bank PSUM tile + one exp pass, but only
    # when the first chunk is 512 wide so the second matmul's output slice
    # starts exactly at the bank boundary (matmul out must not cross banks)
    groups = []
    j = 0
    while j < len(chunks):
        if j + 1 < len(chunks) and chunks[j][1] == 512:
            groups.append(chunks[j : j + 2])
            j += 2
        else:
            groups.append(chunks[j : j + 1])
            j += 1
    return mk, groups


def _strip_layout():
    base = [512 * _c0(i) for i in range(NT)]
    width = [L - b for b in base]
    off = np.concatenate([[0], np.cumsum(width)]).astype(int)
    return base, width, off


def build_nc():
    nc = bacc.Bacc("TRN2", target_bir_lowering=False, debug=False, num_devices=B)

    inp = {}
    for name, shape in [
        ("skills_pt", [H, L]),  # [p, t*128] tile-partitioned natural layout
        ("x_pt", [H, L]),
        ("xT", [H, L]),
        ("indT", [H, L]),
        ("nsT", [H, L]),
        ("mask0", [H, 512]),
        ("maskcd", [H, 256]),
        ("w1_pt", [H, 384]),
        ("w2_pt", [H, 384]),
        ("cw_pt", [H, 384]),
        ("b1", [H, 1]),
        ("b2", [H, 1]),
        ("cb", [H, 1]),
    ]:
        inp[name] = nc.declare_dram_parameter(name, shape, F32, isOutput=False)
    outT = nc.declare_dram_parameter("outT", [H, L], F32, isOutput=True)

    with tile.TileContext(nc) as tc:
        _body(nc, tc, inp, outT)
    nc.compile()
    return nc


def _body(nc, tc, inp, outT):
    base_l, width, off = _strip_layout()

    with (
        tc.tile_pool(name="persist", bufs=1) as P,
        tc.tile_pool(name="small", bufs=2) as SM,
    ):
        # ---- persistent SBUF buffers ----
        # Anything consumed by an FP32r matmul must be *written* as float32r
        # (walrus BIR verifier requirement), so those tiles are F32R-typed.
        sk = P.tile([H, L], F32, name="sk")  # skills, then sn (in place)
        xn = P.tile([H, L], F32R, name="xn")  # x natural, then xs (in place)
        snT = P.tile([H, L], F32R, name="snT")
        strip = P.tile([H, int(off[NT])], F32R, name="strip")  # ET storage
        xTs = P.tile([H, L], F32R, name="xTs")
        indTs = P.tile([H, L], F32R, name="indTs")
        nsTs = P.tile([H, L], F32, name="nsTs")
        x2T = P.tile([H, L], F32R, name="x2T")
        h0 = P.tile([H, L + 2], F32R, name="h0")  # conv ping
        h1 = P.tile([H, L + 2], F32R, name="h1")  # conv pong
        h2 = P.tile([H, L], F32, name="h2")  # conv3 out (full fp32, no pad)
        m0 = P.tile([H, 512], F32, name="m0")
        mcd = P.tile([H, 256], F32, name="mcd")
        w1t = P.tile([H, 3, H], F32R, name="w1t")
        w2t = P.tile([H, 3, H], F32R, name="w2t")
        cwt = P.tile([H, 3, H], F32R, name="cwt")
        b1t = P.tile([H, 1], F32, name="b1t")
        b2t = P.tile([H, 1], F32, name="b2t")
        cbt = P.tile([H, 1], F32, name="cbt")
        ident = P.tile([H, H], F32, name="ident")
        norm2 = P.tile([H, NT], F32, name="norm2")
        inv_n = P.tile([H, NT], F32, name="inv_n")
        ssub = P.tile([H, NT, 8], F32, name="ssub")
        s_inv = P.tile([H, NT], F32, name="s_inv")
        sqs = P.tile([H, H], F32, name="sqs")
        zeros = P.tile([H, 512], F32, name="zeros")

        # ---- input DMAs ----
        # Queue order is priority order (transfers on one queue serialize):
        # skills first on sync (unblocks the whole pipeline), then xn; the
        # late-needed big tensors go behind small ones on the other queues so
        # they don't steal HBM bandwidth from skills.
        for g in range(4):
            nc.sync.dma_start(
                out=sk[:, 512 * g : 512 * (g + 1)],
                in_=inp["skills_pt"][:, 512 * g : 512 * (g + 1)],
            )
        nc.sync.dma_start(out=xn, in_=inp["x_pt"][:, :].bitcast(F32R))
        nc.gpsimd.dma_start(out=m0, in_=inp["mask0"][:, :])
        nc.gpsimd.dma_start(out=mcd, in_=inp["maskcd"][:, :])
        nc.gpsimd.dma_start(out=b1t, in_=inp["b1"][:, :])
        nc.gpsimd.dma_start(out=b2t, in_=inp["b2"][:, :])
        nc.gpsimd.dma_start(out=cbt, in_=inp["cb"][:, :])
        nc.gpsimd.dma_start(out=xTs, in_=inp["xT"][:, :].bitcast(F32R))
        nc.scalar.dma_start(out=w1t, in_=inp["w1_pt"][:, :].rearrange("p (r h) -> p r h", r=3).bitcast(F32R))
        nc.scalar.dma_start(out=w2t, in_=inp["w2_pt"][:, :].rearrange("p (r h) -> p r h", r=3).bitcast(F32R))
        nc.scalar.dma_start(out=cwt, in_=inp["cw_pt"][:, :].rearrange("p (k h) -> p k h", k=3).bitcast(F32R))
        nc.scalar.dma_start(out=indTs, in_=inp["indT"][:, :].bitcast(F32R))
        nc.scalar.dma_start(out=nsTs, in_=inp["nsT"][:, :])

        make_identity(nc, ident)

        # Zero-fill ET strip regions never written by phase 1, and conv pads.
        # Memset can't encode dtype float32r, so stage zeros in an F32 tile
        # and copy (the copy converts and satisfies the FP32r-writer rule).
        nc.vector.memset(zeros, 0.0)
        for i in range(1, NT):
            zf = (128 * i - 128) - base_l[i]
            if zf > 0:
                nc.vector.tensor_copy(
                    out=strip[:, int(off[i]) : int(off[i]) + zf], in_=zeros[:, :zf]
                )
        nc.vector.tensor_copy(out=h0[:, 0:2], in_=zeros[:, 0:2])
        nc.vector.tensor_copy(out=h1[:, 0:2], in_=zeros[:, 0:2])

        # Preload all ACT spline tables with dummy ops while DMAs stream in,
        # so no ~1.3us ACT_TABLE_LOAD lands mid-pipeline.
        if PRELOAD_TABLES:
            tdum = P.tile([H, 1], F32, name="tdum")
            for fn in (AF.Sqrt, AF.Exp, AF.Sigmoid, AF.Relu):
                nc.scalar.activation(out=tdum, in_=zeros[:, 0:1], func=fn)

        with tc.tile_pool(name="ps_work", bufs=2, space="PSUM") as PSW:
            # ---- prologue: norms, sn, snT — pipelined per 4-tile group ----
            for g in range(4):
                for t in range(4 * g, 4 * g + 4):
                    # norm2_t = sum_d skills_t^2 on DVE (keeps ACT free; NB
                    # tensor_tensor_reduce hangs on HW here, so mul+reduce)
                    nc.vector.tensor_mul(
                        out=sqs,
                        in0=sk[:, 128 * t : 128 * (t + 1)],
                        in1=sk[:, 128 * t : 128 * (t + 1)],
                    )
                    nc.vector.reduce_sum(
                        out=norm2[:, t : t + 1], in_=sqs, axis=AX.X
                    )
                gsl = slice(4 * g, 4 * g + 4)
                nc.scalar.activation(out=inv_n[:, gsl], in_=norm2[:, gsl], func=AF.Sqrt)
                nc.vector.reciprocal(out=inv_n[:, gsl], in_=inv_n[:, gsl])
                for t in range(4 * g, 4 * g + 4):
                    nc.vector.tensor_scalar_mul(
                        out=sk[:, 128 * t : 128 * (t + 1)],
                        in0=sk[:, 128 * t : 128 * (t + 1)],
                        scalar1=inv_n[:, t : t + 1],
                    )
                tps = PSW.tile([H, 1024], F32, tag="cos", name=f"tp{g}")
                for j, t in enumerate(range(4 * g, 4 * g + 4)):
                    nc.tensor.transpose(
                        tps[:, 128 * j : 128 * (j + 1)],
                        sk[:, 128 * t : 128 * (t + 1)],
                        ident,
                    )
                nc.vector.tensor_copy(
                    out=snT[:, 512 * g : 512 * (g + 1)], in_=tps[:, 0:512]
                )

            # ---- phase 1: ET strips + colsums ----
            for i in range(NT):
                mi = 128 * i
                lhs = snT[:, mi : mi + 128]
                (mlo, mn, mkind), groups = _p1_chunks(i)
                nacc = 0
                # masked diagonal/corner chunk
                ps = PSW.tile([H, 1024], F32, tag="cos")
                nc.tensor.matmul(
                    ps[:, :mn], lhsT=lhs, rhs=snT[:, mlo : mlo + mn],
                    start=True, stop=True,
                )
                nc.scalar.activation(out=ps[:, :mn], in_=ps[:, :mn], func=AF.Exp)
                mt = m0[:, :mn] if mkind == "mask0" else mcd[:, :mn]
                d0 = int(off[i]) + mlo - base_l[i]
                nc.vector.scalar_tensor_tensor(
                    out=strip[:, d0 : d0 + mn], in0=ps[:, :mn], scalar=1.0, in1=mt,
                    op0=ALU.mult, op1=ALU.mult, accum_out=ssub[:, i, nacc : nacc + 1],
                )
                nacc += 1
                # bulk groups: <=2 matmuls into one PSUM pair, one exp+accum
                for grp in groups:
                    glo = grp[0][0]
                    gn = sum(n for _, n in grp)
                    ps = PSW.tile([H, 1024], F32, tag="cos")
                    for lo, n in grp:
                        nc.tensor.matmul(
                            ps[:, lo - glo : lo - glo + n],
                            lhsT=lhs, rhs=snT[:, lo : lo + n],
                            start=True, stop=True,
                        )
                    d0 = int(off[i]) + glo - base_l[i]
                    nc.scalar.activation(
                        out=strip[:, d0 : d0 + gn], in_=ps[:, :gn], func=AF.Exp,
                        accum_out=ssub[:, i, nacc : nacc + 1],
                    )
                    nacc += 1
                nc.vector.reduce_sum(
                    out=s_inv[:, i : i + 1], in_=ssub[:, i, 0:nacc], axis=AX.X
                )
                nc.vector.reciprocal(out=s_inv[:, i : i + 1], in_=s_inv[:, i : i + 1])
                # xs_i = x_i / S_i (in place)
                nc.vector.tensor_scalar_mul(
                    out=xn[:, mi : mi + 128],
                    in0=xn[:, mi : mi + 128],
                    scalar1=s_inv[:, i : i + 1],
                )

            # ---- phase 2: x2T = sum_i xs_i^T-contract ET_i ----
            with tc.tile_pool(name="ps_x2", bufs=1, space="PSUM") as PSX:
                x2ps = [PSX.tile([H, 512], F32, name=f"x2ps{c}") for c in range(4)]
                for i in range(NT):
                    for c in range(_c0(i), 4):
                        rhs = strip[
                            :, int(off[i]) + 512 * c - base_l[i] : int(off[i]) + 512 * c - base_l[i] + 512
                        ]
                        nc.tensor.matmul(
                            x2ps[c],
                            lhsT=xn[:, 128 * i : 128 * (i + 1)],
                            rhs=rhs,
                            start=(i == 0),
                            stop=(i == _LAST_I[c]),
                        )
                for c in range(4):
                    nc.vector.tensor_copy(out=x2T[:, 512 * c : 512 * (c + 1)], in_=x2ps[c])

        # ---- GLU ----
        with tc.tile_pool(name="ps_glu", bufs=1, space="PSUM") as PSM:
            gps = [PSM.tile([H, 512], F32, name=f"gps{c}") for c in range(4)]
            hps = [PSM.tile([H, 512], F32, name=f"hps{c}") for c in range(4)]
            srcs = [xTs, x2T, indTs]
            # per-chunk so chunk 0's gate/h are ready before chunk 3's matmuls
            # (weights reload per matmul anyway for fp32r, so order is free)
            for c in range(4):
                csl = slice(512 * c, 512 * (c + 1))
                for r in range(3):
                    nc.tensor.matmul(
                        gps[c], lhsT=w1t[:, r, :], rhs=srcs[r][:, csl],
                        start=(r == 0), stop=(r == 2),
                    )
                for r in range(3):
                    nc.tensor.matmul(
                        hps[c], lhsT=w2t[:, r, :], rhs=srcs[r][:, csl],
                        start=(r == 0), stop=(r == 2),
                    )
                gate = SM.tile([H, 512], F32, tag="gate")
                nc.scalar.activation(out=gate, in_=gps[c], func=AF.Sigmoid, bias=b1t)
                # h0 = (hps + b2) * gate
                nc.vector.scalar_tensor_tensor(
                    out=h0[:, 2 + 512 * c : 2 + 512 * (c + 1)],
                    in0=hps[c], scalar=b2t, in1=gate,
                    op0=ALU.add, op1=ALU.mult,
                )

        # ---- convs ----
        with tc.tile_pool(name="ps_cv", bufs=4, space="PSUM") as PSC:
            bufs = [h0, h1, h0]
            for layer in range(3):
                src = bufs[layer]
                for c in range(4):
                    cps = PSC.tile([H, 512], F32, tag="cv")
                    for k in range(3):
                        nc.tensor.matmul(
                            cps, lhsT=cwt[:, k, :],
                            rhs=src[:, 512 * c + k : 512 * c + k + 512],
                            start=(k == 0), stop=(k == 2),
                        )
                    if layer < 2:
                        dsl = bufs[layer + 1][:, 2 + 512 * c : 2 + 512 * (c + 1)]
                        nc.scalar.activation(out=dsl, in_=cps, func=AF.Relu, bias=cbt)
                    else:
                        # last conv: fp32 out, fuse *next_skill and store early
                        csl = slice(512 * c, 512 * (c + 1))
                        nc.scalar.activation(
                            out=h2[:, csl], in_=cps, func=AF.Relu, bias=cbt
                        )
                        nc.vector.tensor_mul(
                            out=h2[:, csl], in0=h2[:, csl], in1=nsTs[:, csl]
                        )
                        if c % 2 == 1:
                            osl = slice(512 * (c - 1), 512 * (c + 1))
                            nc.sync.dma_start(out=outT[:, osl], in_=h2[:, osl])


_NC = None


def _get_nc():
    global _NC
    if _NC is None:
        _NC = build_nc()
    return _NC


def _to_pt(a):
    # [L, H] -> [H(partition = l within tile), NT*H(free: tile-major, then h)]
    return np.ascontiguousarray(
        a.reshape(NT, H, H).transpose(1, 0, 2).reshape(H, L).astype(np.float32)
    )


def _masks():
    mr = np.arange(H)[:, None]
    lc0 = np.arange(512)[None, :]
    mask0 = ((lc0 >= mr - 2) & ((lc0 != mr) | (mr == 0))).astype(np.float32)
    lcc = np.arange(256)[None, :]
    maskcd = ((lcc >= mr + 126) & (lcc != mr + 128)).astype(np.float32)
    return mask0, maskcd


def make_in_maps(x, skills, individual, next_skill,
                 glu_w1, glu_b1, glu_w2, glu_b2, conv_w, conv_b):
    x = np.asarray(x, np.float32)
    skills = np.asarray(skills, np.float32)
    individual = np.asarray(individual, np.float32)
    next_skill = np.asarray(next_skill, np.float32)
    mask0, maskcd = _masks()
    w1_pt = np.ascontiguousarray(
        np.asarray(glu_w1, np.float32).reshape(3, H, H).transpose(1, 0, 2).reshape(H, 384)
    )
    w2_pt = np.ascontiguousarray(
        np.asarray(glu_w2, np.float32).reshape(3, H, H).transpose(1, 0, 2).reshape(H, 384)
    )
    cw_pt = np.ascontiguousarray(
        np.asarray(conv_w, np.float32).transpose(1, 0, 2).reshape(H, 384)
    )
    b1 = np.asarray(glu_b1, np.float32).reshape(H, 1)
    b2 = np.asarray(glu_b2, np.float32).reshape(H, 1)
    cb = np.asarray(conv_b, np.float32).reshape(H, 1)
    in_maps = []
    for b in range(B):
        in_maps.append({
            "skills_pt": _to_pt(skills[b]),
            "x_pt": _to_pt(x[b]),
            "xT": np.ascontiguousarray(x[b].T),
            "indT": np.ascontiguousarray(individual[b].T),
            "nsT": np.ascontiguousarray(next_skill[b].T),
            "mask0": mask0, "maskcd": maskcd,
            "w1_pt": w1_pt, "w2_pt": w2_pt, "cw_pt": cw_pt,
            "b1": b1, "b2": b2, "cb": cb,
        })
    return in_maps


def run(trace=False, **inputs):
    """Run on the 8 NeuronCores; returns (output [B,L,H], BassKernelResults)."""
    nc = _get_nc()
    in_maps = make_in_maps(**inputs)
    res = run_bass_kernel_spmd(nc, in_maps, list(range(B)), trace=trace)
    out = np.stack([np.ascontiguousarray(res.results[b]["outT"].T) for b in range(B)])
    return out.astype(np.float32), res


def kernel(**inputs):
    out, _ = run(trace=False, **inputs)
    return out


# revision 31
# speedup vs baseline: 1.1606x; 1.1606x over previous
"""Fused sparse-attention CNN kernel for TRN2 (8 NeuronCores, batch-parallel).

Per batch b (one per core), with L=2048, H=128:
  cos[l,m] = <s_l, s_m> / (|s_l||s_m|)  masked to band (m <= l+2, diag removed
  except (0,0)); att = softmax over l (per-column normalization);
  x2 = att @ x; GLU over concat([x, x2, individual]); 3x causal conv1d(K=3)
  + relu; times next_skill.

Key structure exploited on-chip:
  - softmax normalizes over full columns m, so att = E / colsum(E) with
    E = exp(masked cos) and x2 = E^T-layout matmul with x pre-scaled by
    1/colsum. No online softmax needed.
  - E is stored transposed, ET[m, l], so colsums are free-axis reductions
    (fused into the exp pass via accum_out) and both big matmuls contract
    over the partition axis naturally.
  - the band mask means tiles with l < m-2 are skipped entirely (~1/3 of
    the [L, L] work) and only diagonal-adjacent tiles need mask multiplies.
  - everything after the cos matmul stays in SBUF; the [L, L] tensor never
    touches HBM.
"""

import numpy as np

import concourse.bacc as bacc
import concourse.bass as bass
import concourse.tile as tile
from concourse import mybir
from concourse.bass_utils import run_bass_kernel_spmd
from concourse.masks import make_identity

L = 2048
H = 128
NT = 16  # l/m tiles of 128
B = 8
F32 = mybir.dt.float32
F32R = mybir.dt.float32r
AF = mybir.ActivationFunctionType
ALU = mybir.AluOpType
AX = mybir.AxisListType

# last m-tile index contributing to each 512-wide l-chunk of x2
_LAST_I = [4, 8, 12, 15]




def _c0(i):
    # first 512-chunk of l covered by m-tile i's ET strip
    return min(3, max(0, (128 * i - 2) // 512))


def _p1_chunks(i):
    """Phase-1 cos-matmul chunks for m-tile i: (masked_chunk, bulk_groups).

    masked_chunk: (lo, n, mask_kind); bulk_groups: list of lists of (lo, n)
    where each group (<=2 adjacent chunks, total <=1024) shares one PSUM
    tile and one exp pass.
    """
    mi = 128 * i
    if i == 0:
        mk = (0, 512, "mask0")
        lo = 512
    else:
        mk = (mi - 128, 256, "maskcd")
        lo = mi + 128
    rem = L - lo
    chunks = []
    while rem > 0:
        n = 384 if rem == 640 else (512 if rem >= 512 else rem)
        chunks.append((lo, n))
        lo += n
        rem -= n
    # pair adjacent chunks into one 2-bank PSUM tile + one exp pass, but only
    # when the first chunk is 512 wide so the second matmul's output slice
    # starts exactly at the bank boundary (matmul out must not cross banks)
    groups = []
    j = 0
    while j < len(chunks):
        if j + 1 < len(chunks) and chunks[j][1] == 512:
            groups.append(chunks[j : j + 2])
            j += 2
        else:
            groups.append(chunks[j : j + 1])
            j += 1
    return mk, groups


def _strip_layout():
    base = [512 * _c0(i) for i in range(NT)]
    width = [L - b for b in base]
    off = np.concatenate([[0], np.cumsum(width)]).astype(int)
    return base, width, off


def build_nc():
    nc = bacc.Bacc("TRN2", target_bir_lowering=False, debug=False, num_devices=B)

    inp = {}
    for name, shape in [
        ("skills_pt", [H, L]),  # [p, t*128] tile-partitioned natural layout
        ("x_pt", [H, L]),
        ("xT", [H, L]),
        ("indT", [H, L]),
        ("nsT", [H, L]),
        ("mask0", [H, 512]),
        ("maskcd", [H, 256]),
        ("w1_pt", [H, 384]),
        ("w2_pt", [H, 384]),
        ("cw_pt", [H, 384]),
        ("b1", [H, 1]),
        ("b2", [H, 1]),
        ("cb", [H, 1]),
    ]:
        inp[name] = nc.declare_dram_parameter(name, shape, F32, isOutput=False)
    outT = nc.declare_dram_parameter("outT", [H, L], F32, isOutput=True)

    with tile.TileContext(nc) as tc:
        _body(nc, tc, inp, outT)
    nc.compile()
    return nc


def _body(nc, tc, inp, outT):
    base_l, width, off = _strip_layout()

    with (
        tc.tile_pool(name="persist", bufs=1) as P,
        tc.tile_pool(name="small", bufs=2) as SM,
    ):
        # ---- persistent SBUF buffers ----
        # Anything consumed by an FP32r matmul must be *written* as float32r
        # (walrus BIR verifier requirement), so those tiles are F32R-typed.
        sk = P.tile([H, L], F32, name="sk")  # skills, then sn (in place)
        xn = P.tile([H, L], F32R, name="xn")  # x natural, then xs (in place)
        snT = P.tile([H, L], F32R, name="snT")
        strip = P.tile([H, int(off[NT])], F32R, name="strip")  # ET storage
        xTs = P.tile([H, L], F32R, name="xTs")
        indTs = P.tile([H, L], F32R, name="indTs")
        nsTs = P.tile([H, L], F32, name="nsTs")
        x2T = P.tile([H, L], F32R, name="x2T")
        h0 = P.tile([H, L + 2], F32R, name="h0")  # conv ping
        h1 = P.tile([H, L + 2], F32R, name="h1")  # conv pong
        h2 = P.tile([H, L], F32, name="h2")  # conv3 out (full fp32, no pad)
        m0 = P.tile([H, 512], F32, name="m0")
        mcd = P.tile([H, 256], F32, name="mcd")
        w1t = P.tile([H, 3, H], F32R, name="w1t")
        w2t = P.tile([H, 3, H], F32R, name="w2t")
        cwt = P.tile([H, 3, H], F32R, name="cwt")
        b1t = P.tile([H, 1], F32, name="b1t")
        b2t = P.tile([H, 1], F32, name="b2t")
        cbt = P.tile([H, 1], F32, name="cbt")
        ident = P.tile([H, H], F32, name="ident")
        norm2 = P.tile([H, NT], F32, name="norm2")
        inv_n = P.tile([H, NT], F32, name="inv_n")
        ssub = P.tile([H, NT, 8], F32, name="ssub")
        s_inv = P.tile([H, NT], F32, name="s_inv")
        sqs = P.tile([H, 512], F32, name="sqs")
        zeros = P.tile([H, 512], F32, name="zeros")

        # ---- input DMAs ----
        # Each HWDGE queue moves only ~50-75 GB/s here, so (a) skills — which
        # gates the whole pipeline — is striped across all three queues, and
        # (b) queue order is priority order (transfers on one queue serialize).
        qs = [nc.sync, nc.gpsimd, nc.scalar, nc.sync]
        for g in range(4):
            qs[g].dma_start(
                out=sk[:, 512 * g : 512 * (g + 1)],
                in_=inp["skills_pt"][:, 512 * g : 512 * (g + 1)],
            )
        nc.sync.dma_start(out=xn, in_=inp["x_pt"][:, :].bitcast(F32R))
        nc.gpsimd.dma_start(out=m0, in_=inp["mask0"][:, :])
        nc.gpsimd.dma_start(out=mcd, in_=inp["maskcd"][:, :])
        nc.gpsimd.dma_start(out=b1t, in_=inp["b1"][:, :])
        nc.gpsimd.dma_start(out=b2t, in_=inp["b2"][:, :])
        nc.gpsimd.dma_start(out=cbt, in_=inp["cb"][:, :])
        nc.gpsimd.dma_start(out=xTs, in_=inp["xT"][:, :].bitcast(F32R))
        nc.scalar.dma_start(out=w1t, in_=inp["w1_pt"][:, :].rearrange("p (r h) -> p r h", r=3).bitcast(F32R))
        nc.scalar.dma_start(out=w2t, in_=inp["w2_pt"][:, :].rearrange("p (r h) -> p r h", r=3).bitcast(F32R))
        nc.scalar.dma_start(out=cwt, in_=inp["cw_pt"][:, :].rearrange("p (k h) -> p k h", k=3).bitcast(F32R))
        nc.scalar.dma_start(out=indTs, in_=inp["indT"][:, :].bitcast(F32R))
        nc.scalar.dma_start(out=nsTs, in_=inp["nsT"][:, :])

        make_identity(nc, ident)

        # Zero-fill ET strip regions never written by phase 1, and conv pads.
        # Memset can't encode dtype float32r, so stage zeros in an F32 tile
        # and copy (the copy converts and satisfies the FP32r-writer rule).
        nc.vector.memset(zeros, 0.0)
        for i in range(1, NT):
            zf = (128 * i - 128) - base_l[i]
            if zf > 0:
                nc.vector.tensor_copy(
                    out=strip[:, int(off[i]) : int(off[i]) + zf], in_=zeros[:, :zf]
                )
        nc.vector.tensor_copy(out=h0[:, 0:2], in_=zeros[:, 0:2])
        nc.vector.tensor_copy(out=h1[:, 0:2], in_=zeros[:, 0:2])

        with tc.tile_pool(name="ps_work", bufs=2, space="PSUM") as PSW:
            # ---- prologue: norms, sn, snT ----
            # Squares per 4-tile group in two wide DVE ops; ONE Sqrt for all
            # 16 norms so the Sqrt table set never interleaves with Exp's
            # (each ACT table switch costs ~1.3us).
            for g in range(4):
                gsl = slice(512 * g, 512 * (g + 1))
                nc.vector.tensor_mul(out=sqs, in0=sk[:, gsl], in1=sk[:, gsl])
                nc.vector.reduce_sum(
                    out=norm2[:, 4 * g : 4 * (g + 1)],
                    in_=sqs.rearrange("p (t d) -> p t d", d=128),
                    axis=AX.X,
                )
            nc.scalar.activation(out=inv_n, in_=norm2, func=AF.Sqrt)
            nc.vector.reciprocal(out=inv_n, in_=inv_n)
            for g in range(4):
                for t in range(4 * g, 4 * g + 4):
                    nc.vector.tensor_scalar_mul(
                        out=sk[:, 128 * t : 128 * (t + 1)],
                        in0=sk[:, 128 * t : 128 * (t + 1)],
                        scalar1=inv_n[:, t : t + 1],
                    )
                tps = PSW.tile([H, 1024], F32, tag="cos", name=f"tp{g}")
                for j, t in enumerate(range(4 * g, 4 * g + 4)):
                    nc.tensor.transpose(
                        tps[:, 128 * j : 128 * (j + 1)],
                        sk[:, 128 * t : 128 * (t + 1)],
                        ident,
                    )
                nc.vector.tensor_copy(
                    out=snT[:, 512 * g : 512 * (g + 1)], in_=tps[:, 0:512]
                )

            # ---- phase 1: ET strips + colsums ----
            for i in range(NT):
                mi = 128 * i
                lhs = snT[:, mi : mi + 128]
                (mlo, mn, mkind), groups = _p1_chunks(i)
                nacc = 0
                # masked diagonal/corner chunk
                ps = PSW.tile([H, 1024], F32, tag="cos")
                nc.tensor.matmul(
                    ps[:, :mn], lhsT=lhs, rhs=snT[:, mlo : mlo + mn],
                    start=True, stop=True,
                )
                nc.scalar.activation(out=ps[:, :mn], in_=ps[:, :mn], func=AF.Exp)
                mt = m0[:, :mn] if mkind == "mask0" else mcd[:, :mn]
                d0 = int(off[i]) + mlo - base_l[i]
                nc.vector.scalar_tensor_tensor(
                    out=strip[:, d0 : d0 + mn], in0=ps[:, :mn], scalar=1.0, in1=mt,
                    op0=ALU.mult, op1=ALU.mult, accum_out=ssub[:, i, nacc : nacc + 1],
                )
                nacc += 1
                # bulk groups: <=2 matmuls into one PSUM pair, one exp+accum
                for grp in groups:
                    glo = grp[0][0]
                    gn = sum(n for _, n in grp)
                    ps = PSW.tile([H, 1024], F32, tag="cos")
                    for lo, n in grp:
                        nc.tensor.matmul(
                            ps[:, lo - glo : lo - glo + n],
                            lhsT=lhs, rhs=snT[:, lo : lo + n],
                            start=True, stop=True,
                        )
                    d0 = int(off[i]) + glo - base_l[i]
                    nc.scalar.activation(
                        out=strip[:, d0 : d0 + gn], in_=ps[:, :gn], func=AF.Exp,
                        accum_out=ssub[:, i, nacc : nacc + 1],
                    )
                    nacc += 1
                nc.vector.reduce_sum(
                    out=s_inv[:, i : i + 1], in_=ssub[:, i, 0:nacc], axis=AX.X
                )
                nc.vector.reciprocal(out=s_inv[:, i : i + 1], in_=s_inv[:, i : i + 1])
                # xs_i = x_i / S_i (in place)
                nc.vector.tensor_scalar_mul(
                    out=xn[:, mi : mi + 128],
                    in0=xn[:, mi : mi + 128],
                    scalar1=s_inv[:, i : i + 1],
                )

            # ---- phase 2: x2T = sum_i xs_i^T-contract ET_i ----
            with tc.tile_pool(name="ps_x2", bufs=1, space="PSUM") as PSX:
                x2ps = [PSX.tile([H, 512], F32, name=f"x2ps{c}") for c in range(4)]
                for i in range(NT):
                    for c in range(_c0(i), 4):
                        rhs = strip[
                            :, int(off[i]) + 512 * c - base_l[i] : int(off[i]) + 512 * c - base_l[i] + 512
                        ]
                        nc.tensor.matmul(
                            x2ps[c],
                            lhsT=xn[:, 128 * i : 128 * (i + 1)],
                            rhs=rhs,
                            start=(i == 0),
                            stop=(i == _LAST_I[c]),
                        )
                for c in range(4):
                    nc.vector.tensor_copy(out=x2T[:, 512 * c : 512 * (c + 1)], in_=x2ps[c])

        # ---- GLU ----
        with tc.tile_pool(name="ps_glu", bufs=1, space="PSUM") as PSM:
            gps = [PSM.tile([H, 512], F32, name=f"gps{c}") for c in range(4)]
            hps = [PSM.tile([H, 512], F32, name=f"hps{c}") for c in range(4)]
            srcs = [xTs, x2T, indTs]
            # per-chunk so chunk 0's gate/h are ready before chunk 3's matmuls
            # (weights reload per matmul anyway for fp32r, so order is free)
            for c in range(4):
                csl = slice(512 * c, 512 * (c + 1))
                for r in range(3):
                    nc.tensor.matmul(
                        gps[c], lhsT=w1t[:, r, :], rhs=srcs[r][:, csl],
                        start=(r == 0), stop=(r == 2),
                    )
                for r in range(3):
                    nc.tensor.matmul(
                        hps[c], lhsT=w2t[:, r, :], rhs=srcs[r][:, csl],
                        start=(r == 0), stop=(r == 2),
                    )
                gate = SM.tile([H, 512], F32, tag="gate")
                nc.scalar.activation(out=gate, in_=gps[c], func=AF.Sigmoid, bias=b1t)
                # h0 = (hps + b2) * gate
                nc.vector.scalar_tensor_tensor(
                    out=h0[:, 2 + 512 * c : 2 + 512 * (c + 1)],
                    in0=hps[c], scalar=b2t, in1=gate,
                    op0=ALU.add, op1=ALU.mult,
                )

        # ---- convs ----
        with tc.tile_pool(name="ps_cv", bufs=4, space="PSUM") as PSC:
            bufs = [h0, h1, h0]
            for layer in range(3):
                src = bufs[layer]
                for c in range(4):
                    cps = PSC.tile([H, 512], F32, tag="cv")
                    for k in range(3):
                        nc.tensor.matmul(
                            cps, lhsT=cwt[:, k, :],
                            rhs=src[:, 512 * c + k : 512 * c + k + 512],
                            start=(k == 0), stop=(k == 2),
                        )
                    if layer < 2:
                        dsl = bufs[layer + 1][:, 2 + 512 * c : 2 + 512 * (c + 1)]
                        nc.scalar.activation(out=dsl, in_=cps, func=AF.Relu, bias=cbt)
                    else:
                        # last conv: fp32 out, fuse *next_skill and store early
                        csl = slice(512 * c, 512 * (c + 1))
                        nc.scalar.activation(
                            out=h2[:, csl], in_=cps, func=AF.Relu, bias=cbt
                        )
                        nc.vector.tensor_mul(
                            out=h2[:, csl], in0=h2[:, csl], in1=nsTs[:, csl]
                        )
                        oq = [nc.sync, nc.gpsimd, nc.scalar, nc.sync][c]
                        oq.dma_start(out=outT[:, csl], in_=h2[:, csl])


_NC = None


def _get_nc():
    global _NC
    if _NC is None:
        _NC = build_nc()
    return _NC


def _to_pt(a):
    # [L, H] -> [H(partition = l within tile), NT*H(free: tile-major, then h)]
    return np.ascontiguousarray(
        a.reshape(NT, H, H).transpose(1, 0, 2).reshape(H, L).astype(np.float32)
    )


def _masks():
    mr = np.arange(H)[:, None]
    lc0 = np.arange(512)[None, :]
    mask0 = ((lc0 >= mr - 2) & ((lc0 != mr) | (mr == 0))).astype(np.float32)
    lcc = np.arange(256)[None, :]
    maskcd = ((lcc >= mr + 126) & (lcc != mr + 128)).astype(np.float32)
    return mask0, maskcd


def make_in_maps(x, skills, individual, next_skill,
                 glu_w1, glu_b1, glu_w2, glu_b2, conv_w, conv_b):
    x = np.asarray(x, np.float32)
    skills = np.asarray(skills, np.float32)
    individual = np.asarray(individual, np.float32)
    next_skill = np.asarray(next_skill, np.float32)
    mask0, maskcd = _masks()
    w1_pt = np.ascontiguousarray(
        np.asarray(glu_w1, np.float32).reshape(3, H, H).transpose(1, 0, 2).reshape(H, 384)
    )
    w2_pt = np.ascontiguousarray(
        np.asarray(glu_w2, np.float32).reshape(3, H, H).transpose(1, 0, 2).reshape(H, 384)
    )
    cw_pt = np.ascontiguousarray(
        np.asarray(conv_w, np.float32).transpose(1, 0, 2).reshape(H, 384)
    )
    b1 = np.asarray(glu_b1, np.float32).reshape(H, 1)
    b2 = np.asarray(glu_b2, np.float32).reshape(H, 1)
    cb = np.asarray(conv_b, np.float32).reshape(H, 1)
    in_maps = []
    for b in range(B):
        in_maps.append({
            "skills_pt": _to_pt(skills[b]),
            "x_pt": _to_pt(x[b]),
            "xT": np.ascontiguousarray(x[b].T),
            "indT": np.ascontiguousarray(individual[b].T),
            "nsT": np.ascontiguousarray(next_skill[b].T),
            "mask0": mask0, "maskcd": maskcd,
            "w1_pt": w1_pt, "w2_pt": w2_pt, "cw_pt": cw_pt,
            "b1": b1, "b2": b2, "cb": cb,
        })
    return in_maps


def run(trace=False, **inputs):
    """Run on the 8 NeuronCores; returns (output [B,L,H], BassKernelResults)."""
    nc = _get_nc()
    in_maps = make_in_maps(**inputs)
    res = run_bass_kernel_spmd(nc, in_maps, list(range(B)), trace=trace)
    out = np.stack([np.ascontiguousarray(res.results[b]["outT"].T) for b in range(B)])
    return out.astype(np.float32), res


def kernel(**inputs):
    out, _ = run(trace=False, **inputs)
    return out
